# revision 19
# baseline (speedup 1.0000x reference)
"""Causal self-attention (B=2, S=2048, D=1024, H=16) on 8 TRN2 NeuronCores.

Sharding: tensor-parallel over heads (2 heads/core) for qkv+attention,
then chunked AllToAll to token-parallel (512 tokens/core) for the output
projection.

Per-core kernel (SPMD, identical program, per-core weight slices as inputs):
  1. qkv^T projection from host-pre-transposed, host-bf16-cast x^T:
       Q^T/K^T/V^T [128ch(2 heads x 64), 4096 tok] = W_slice^T @ x
     (bf16 matmuls, fp32 PSUM accumulation)
  2. V^T -> V via batched 128x128 PE transposes (both heads per transpose),
     ones column interleaved per head so AV also produces the softmax
     denominator.
  3. Attention as one flat (b, qc, kc) software pipeline:
     - score matmuls for the two local heads are emitted back-to-back; they
       occupy disjoint row-halves of the PE array (tile_position auto-derived
       from kT partition base 0/64) so they run CONCURRENTLY (2x).
     - exp on ScalarE with scale=1/sqrt(hd) fused; causal = block skipping +
       one triangle-mask multiply on diagonal blocks.
     - AV accumulates y^T (64 rows) + denominator row (ones col) in PSUM.
     - sender-side normalization: reciprocal of den row, broadcast via a
       col-packed pair of ones-matmuls (one PSUM bank, halves at partitions
       0:64/64:128), fused into the f32->bf16 cast of y.
  4. AllToAll in FOUR bf16 chunks (one per (batch, qc-pair)), each issued as
     soon as its attention slice is done -> overlapped with later attention.
  5. Output projection per chunk (128 tokens), interleaved between attention
     chunks so the PE stays dense/warm; out rows are g-major (g = 2b+qh).
Host gathers the 8 token-slices and reorders (see _gather).
"""

import numpy as np
from contextlib import ExitStack

import concourse.bass as bass
import concourse.bacc as bacc
import concourse.tile as tile
from concourse import mybir
from concourse.bass_utils import run_bass_kernel_spmd
from concourse.masks import make_identity

B, S, D = 2, 2048, 1024
H, HD = 16, 64
NCORE = 8
HPC = H // NCORE          # heads per core = 2
CW = HPC * HD             # channels per core = 128
T = B * S                 # 4096 tokens
TPC = T // NCORE          # 512 tokens per core (proj phase)
TCH = 512                 # token chunk for qkv projection
NT = T // TCH             # 8
QCH = 512                 # query chunk
KCH = 128                 # key chunk
NQC = S // QCH            # 4 query chunks per batch
NKC = S // KCH            # 16 key chunks per batch
DK = D // 128             # 8 contraction chunks of 128
NG = 4                    # A2A chunks: (b, qh)
GT = 128                  # tokens per core per chunk

f32 = mybir.dt.float32
f32r = mybir.dt.float32r
bf16 = mybir.dt.bfloat16
AF = mybir.ActivationFunctionType


def _build():
    nc = bacc.Bacc(None, target_bir_lowering=False, num_devices=NCORE)

    xT = nc.dram_tensor("xT", [128, DK, T], bf16, kind="ExternalInput")
    wq = nc.dram_tensor("wq", [128, DK, CW], bf16, kind="ExternalInput")
    wk = nc.dram_tensor("wk", [128, DK, CW], bf16, kind="ExternalInput")
    wv = nc.dram_tensor("wv", [128, DK, CW], bf16, kind="ExternalInput")
    bqkv = nc.dram_tensor("bqkv", [3, CW], f32, kind="ExternalInput")
    wp = nc.dram_tensor("wp", [128, DK, D], bf16, kind="ExternalInput")
    bp = nc.dram_tensor("bp", [1, D], f32, kind="ExternalInput")
    out = nc.dram_tensor("out", [TPC, D], f32, kind="ExternalOutput")

    with ExitStack() as ctx:
        tc = ctx.enter_context(tile.TileContext(nc))
        const = ctx.enter_context(tc.tile_pool(name="const", bufs=1))
        dram = ctx.enter_context(tc.tile_pool(name="dram", bufs=1, space="DRAM"))
        wqkv_pool = ctx.enter_context(tc.tile_pool(name="wqkv", bufs=1))
        xt_pool = ctx.enter_context(tc.tile_pool(name="xt", bufs=3))
        qkvt_pool = ctx.enter_context(tc.tile_pool(name="qkvt", bufs=1))
        wp_pool = ctx.enter_context(tc.tile_pool(name="wpp", bufs=1))
        vpool = ctx.enter_context(tc.tile_pool(name="vpool", bufs=2))
        ppool = ctx.enter_context(tc.tile_pool(name="ppool", bufs=8))
        ypool = ctx.enter_context(tc.tile_pool(name="ypool", bufs=4))
        rpool = ctx.enter_context(tc.tile_pool(name="rpool", bufs=4))
        rgpool = ctx.enter_context(tc.tile_pool(name="rgpool", bufs=2))
        opool = ctx.enter_context(tc.tile_pool(name="opool", bufs=2))
        ps_big = ctx.enter_context(tc.tile_pool(name="ps_big", bufs=2, space="PSUM"))
        ps_sc = ctx.enter_context(tc.tile_pool(name="ps_sc", bufs=2, space="PSUM"))
        ps_y = ctx.enter_context(tc.tile_pool(name="ps_y", bufs=2, space="PSUM"))

        # ---- constants ----
        identity = const.tile([128, 128], bf16)
        make_identity(nc, identity[:])
        # mask[k, q] = 1.0 if k <= q else 0.0  (keep lower-left in S^T layout)
        mask = const.tile([128, 128], bf16)
        nc.gpsimd.memset(mask[:], 0.0)
        nc.gpsimd.affine_select(
            out=mask[:], in_=mask[:],
            compare_op=mybir.AluOpType.is_ge,  # iota(k-q-1) >= 0 (k>q) -> keep 0; else fill 1
            fill=1.0, base=-1, pattern=[[-1, 128]], channel_multiplier=1,
        )
        ones_f32 = const.tile([128, 128], f32)
        nc.vector.memset(ones_f32[:], 1.0)
        ones_bf = const.tile([128, 32], bf16)
        nc.vector.memset(ones_bf[:], 1.0)
        ones_row = const.tile([1, 128], f32r)
        nc.vector.tensor_copy(ones_row[:], ones_f32[0:1, :])
        bias_sb = const.tile([128, 3], f32)
        nc.sync.dma_start(bias_sb[:], bqkv[:].rearrange("g p -> p g"))
        bp_sb = const.tile([1, D], f32r)
        nc.sync.dma_start(bp_sb[:], bp[:].bitcast(f32r))

        # ---- weights (bf16) ----
        wq_sb = wqkv_pool.tile([128, DK, CW], bf16)
        wk_sb = wqkv_pool.tile([128, DK, CW], bf16)
        wv_sb = wqkv_pool.tile([128, DK, CW], bf16)
        nc.sync.dma_start(wq_sb[:], wq[:])
        nc.sync.dma_start(wk_sb[:], wk[:])
        nc.sync.dma_start(wv_sb[:], wv[:])

        # ---- A2A buffers (bf16, 4 chunks of (b, qh)) ----
        SH = CW + 2  # 128 y rows (2 heads) + 2 denominator rows
        # b0: two (qh) chunks of 128 tok/core; b1: four per-qc chunks of
        # 64 tok/core (smaller final A2A -> shorter tail)
        send_g = [dram.tile([NCORE, SH, GT], bf16, name=f"send{g}") for g in range(2)]
        recv_g = [dram.tile([NCORE, SH, GT], bf16, name=f"recv{g}") for g in range(2)]
        send_q = [dram.tile([NCORE, SH, GT // 2], bf16, name=f"sendq{q}") for q in range(NQC)]
        recv_q = [dram.tile([NCORE, SH, GT // 2], bf16, name=f"recvq{q}") for q in range(NQC)]

        # ---- phase 2: batched V transposes (both heads per 128x128 block),
        # ones column interleaved per head (col 64 of each 65-col half) ----
        vboths = {}

        def emit_vtrans(b):
            vboth = vpool.tile([128, NKC, 2 * (HD + 1)], bf16, name=f"vboth{b}")
            vboths[b] = vboth
            nc.vector.tensor_copy(
                vboth[:].rearrange("p c (h x) -> p c h x", x=HD + 1)[:, :, :, HD],
                ones_bf[:].rearrange("p (c h) -> p c h", h=2),
            )
            VG = 4  # transposes batched per PSUM bank
            for g in range(NKC // VG):
                pst = ps_big.tile([128, VG, 128], bf16, tag="psbig")
                for u in range(VG):
                    kc = g * VG + u
                    nc.tensor.transpose(
                        pst[:, u, :],
                        vT[:, b * S + kc * KCH: b * S + (kc + 1) * KCH],
                        identity[:],
                    )
                nc.vector.tensor_copy(
                    vboth[:, g * VG:(g + 1) * VG, :]
                    .rearrange("p c (h x) -> p c h x", x=HD + 1)[:, :, :, 0:HD],
                    pst[:].rearrange("p c (h x) -> p c h x", x=HD),
                )

        # warmup collective on scratch (contents unused): absorbs NRT's
        # ~11.5us first-trigger latency + slow first transfer under qkv
        warm_s = dram.tile([NCORE, 1024], bf16, name="warm_s")
        warm_r = dram.tile([NCORE, 1024], bf16, name="warm_r")
        nc.gpsimd.collective_compute(
            "AllToAll", mybir.AluOpType.bypass,
            replica_groups=[list(range(NCORE))],
            ins=[warm_s[:].opt()], outs=[warm_r[:].opt()],
        )

        # ---- phase 1: qkv^T projection ----
        qT = qkvt_pool.tile([128, T], bf16)
        kT = qkvt_pool.tile([128, T], bf16)
        vT = qkvt_pool.tile([128, T], bf16)
        wp_sb = wp_pool.tile([128, DK, D], bf16)
        for tp in range(NT // 2):
            t0, t1 = 2 * tp, 2 * tp + 1
            xt = xt_pool.tile([128, DK, 2 * TCH], bf16, tag="xt")
            xr = xT[:]
            hk = DK // 2
            for ci in range(2):
                cs = slice(ci * hk, (ci + 1) * hk)
                nc.sync.dma_start(xt[:, cs, 0:TCH], xr[:, cs, t0 * TCH:(t0 + 1) * TCH])
                nc.sync.dma_start(xt[:, cs, TCH:2 * TCH], xr[:, cs, t1 * TCH:(t1 + 1) * TCH])
            if tp == 1:
                # wp needed only at proj; don't let it delay the first xt chunks
                nc.sync.dma_start(wp_sb[:], wp[:])
            for gi, (wsb, dst) in enumerate([(wq_sb, qT), (wk_sb, kT), (wv_sb, vT)]):
                ps0 = ps_big.tile([128, TCH], f32, tag="psbig")
                ps1 = ps_big.tile([128, TCH], f32, tag="psbig")
                for c in range(DK):
                    # consecutive matmuls share lhsT -> one weight load serves two
                    nc.tensor.matmul(
                        ps0[:], lhsT=wsb[:, c, :], rhs=xt[:, c, 0:TCH],
                        start=(c == 0), stop=(c == DK - 1),
                    )
                    nc.tensor.matmul(
                        ps1[:], lhsT=wsb[:, c, :], rhs=xt[:, c, TCH:2 * TCH],
                        start=(c == 0), stop=(c == DK - 1),
                    )
                for ti, ps in ((t0, ps0), (t1, ps1)):
                    nc.vector.tensor_scalar_add(
                        dst[:, ti * TCH:(ti + 1) * TCH], ps[:], bias_sb[:, gi:gi + 1])
            if t1 * TCH + TCH == S:
                emit_vtrans(0)
            elif t1 * TCH + TCH == 2 * S:
                emit_vtrans(1)

        # ---- phase 3+4+5: flat attention pipeline, chunked A2A, interleaved proj ----
        def geom(qc, kc):
            q0 = qc * QCH
            diag = kc * KCH >= q0
            koff = kc * KCH - q0 if diag else 0
            return koff, QCH - koff

        # flat step list: (b, qc, kc)
        steps = []
        for b in range(B):
            for qc in range(NQC):
                nkc = 4 * (qc + 1)
                for kc in range(nkc):
                    steps.append((b, qc, kc))

        ypss = {}   # (b, qc) -> [yps_h0, yps_h1]
        pts = {}    # (b, qc, kc, hl) -> pt tile

        def emit_scores(b, qc, kc):
            koff, W_ = geom(qc, kc)
            # both heads' scores into one 2-bank PSUM tile: the score MMs stay
            # adjacent (concurrent row-halves of the PE) and ONE ACTIVATE
            # covers both heads (halves the per-call exp overhead)
            sps = ps_sc.tile([128, HPC, QCH], f32, tag="sps")
            for hl in range(HPC):
                r0 = hl * HD
                nc.tensor.matmul(
                    sps[:, hl, 0:W_],
                    lhsT=kT[r0:r0 + HD, b * S + kc * KCH: b * S + (kc + 1) * KCH],
                    rhs=qT[r0:r0 + HD, b * S + qc * QCH + koff: b * S + (qc + 1) * QCH],
                    start=True, stop=True,
                )
            pt = ppool.tile([128, HPC, QCH], bf16, tag="pt")
            nc.scalar.activation(pt[:, :, 0:W_], sps[:, :, 0:W_], AF.Exp, scale=0.125)
            if kc * KCH >= qc * QCH:
                for hl in range(HPC):
                    nc.vector.tensor_mul(pt[:, hl, 0:KCH], pt[:, hl, 0:KCH], mask[:])
            pts[(b, qc, kc)] = pt

        def emit_av(b, qc, kc):
            koff, W_ = geom(qc, kc)
            nkc = 4 * (qc + 1)
            if kc == 0:
                ypss[(b, qc)] = [
                    ps_y.tile([HD + 1, QCH], f32, tag="yps", name=f"yps{b}{qc}{hl}")
                    for hl in range(HPC)
                ]
            vboth = vboths[b]
            last = kc == nkc - 1
            pt = pts.pop((b, qc, kc))
            for hl in range(HPC):
                nc.tensor.matmul(
                    ypss[(b, qc)][hl][:, koff:QCH],
                    lhsT=vboth[:, kc, hl * (HD + 1):(hl + 1) * (HD + 1)],
                    rhs=pt[:, hl, 0:W_],
                    start=(kc == 0), stop=last,
                )
            if last:
                # ship unnormalized y (bf16) + denominator rows; the receiver
                # normalizes (recip is cheap there: [16,128] once per chunk)
                yp = ypss.pop((b, qc))
                for hl in range(HPC):
                    ysb = ypool.tile([HD + 1, QCH], bf16, tag="ysb", bufs=8)
                    nc.vector.tensor_copy(ysb[:], yp[hl][:])
                    # SBUF AP stays partition-major: transpose traversal on
                    # the DRAM side
                    if b == 0:
                        g = qc // 2
                        j0 = (qc % 2) * 4
                        nc.sync.dma_start(
                            send_g[g][j0:j0 + 4, hl * HD:(hl + 1) * HD, :]
                            .rearrange("j r t -> r j t"),
                            ysb[0:HD, :].rearrange("r (j t) -> r j t", t=GT),
                        )
                        nc.sync.dma_start(
                            send_g[g][j0:j0 + 4, CW + hl:CW + hl + 1, :]
                            .rearrange("j r t -> r j t"),
                            ysb[HD:HD + 1, :].rearrange("r (j t) -> r j t", t=GT),
                        )
                    else:
                        nc.sync.dma_start(
                            send_q[qc][:, hl * HD:(hl + 1) * HD, :]
                            .rearrange("j r t -> r j t"),
                            ysb[0:HD, :].rearrange("r (j t) -> r j t", t=GT // 2),
                        )
                        nc.sync.dma_start(
                            send_q[qc][:, CW + hl:CW + hl + 1, :]
                            .rearrange("j r t -> r j t"),
                            ysb[HD:HD + 1, :].rearrange("r (j t) -> r j t", t=GT // 2),
                        )

        def emit_a2a(snd, rcv):
            nc.gpsimd.collective_compute(
                "AllToAll", mybir.AluOpType.bypass,
                replica_groups=[list(range(NCORE))],
                ins=[snd[:].opt()], outs=[rcv[:].opt()],
            )

        def emit_proj(g, recvs):
            HT = GT // len(recvs)
            rg = rgpool.tile([128, NCORE, GT], bf16, tag="rg")
            dden = rgpool.tile([2 * NCORE, GT], bf16, tag="dden")
            for p, rv in enumerate(recvs):
                nc.sync.dma_start(rg[:, :, p * HT:(p + 1) * HT],
                                  rv[:, 0:CW, :].rearrange("c p t -> p c t"))
                nc.sync.dma_start(dden[0:NCORE, p * HT:(p + 1) * HT], rv[:, CW, :])
                nc.sync.dma_start(dden[NCORE:2 * NCORE, p * HT:(p + 1) * HT],
                                  rv[:, CW + 1, :])
            rcp = rgpool.tile([2 * NCORE, GT], f32, tag="rcp")
            nc.vector.reciprocal(rcp[:], dden[:])
            # fold the 16 recip rows into one partition's free dim, then
            # broadcast to all 128 partitions in one gpsimd op
            rcpf = rgpool.tile([1, 2 * NCORE, GT], f32, tag="rcpf")
            nc.sync.dma_start(rcpf[:], rcp[:])
            sclb = rgpool.tile([128, 2 * NCORE, GT], f32, tag="sclb")
            nc.gpsimd.partition_broadcast(sclb[:], rcpf[0:1, :, :])
            rgn = rgpool.tile([128, NCORE, GT], bf16, tag="rgn")
            sv = sclb[:].rearrange("p (h c) t -> p h c t", c=NCORE)
            for hl in range(HPC):
                nc.vector.tensor_mul(
                    rgn[hl * HD:(hl + 1) * HD, :, :],
                    rg[hl * HD:(hl + 1) * HD, :, :],
                    sv[hl * HD:(hl + 1) * HD, hl, :, :],
                )
            pss = [ps_big.tile([128, 512], f32, tag="psbig", name=f"pso{g}{n}")
                   for n in range(D // 512)]
            for c in range(DK):
                for n in range(D // 512):
                    nc.tensor.matmul(
                        pss[n][:],
                        lhsT=rgn[:, c, :],
                        rhs=wp_sb[:, c, n * 512:(n + 1) * 512],
                        start=(c == 0), stop=False,
                    )
            for n in range(D // 512):
                nc.tensor.matmul(
                    pss[n][:], lhsT=ones_row[:],
                    rhs=bp_sb[:, n * 512:(n + 1) * 512],
                    start=False, stop=True,
                )
                osb = opool.tile([128, 512], f32, tag="osb")
                nc.vector.tensor_copy(osb[:], pss[n][:])
                nc.sync.dma_start(out[g * GT:(g + 1) * GT, n * 512:(n + 1) * 512], osb[:])

        LOOK = 2
        nstep = len(steps)
        # A2A fires at each chunk's last step; proj#g emitted once its A2A
        # should have landed (proj#3 at the end)
        a2a_at = {11: (send_g[0], recv_g[0]), 39: (send_g[1], recv_g[1]),
                  43: (send_q[0], recv_q[0]), 51: (send_q[1], recv_q[1]),
                  63: (send_q[2], recv_q[2]), 79: (send_q[3], recv_q[3])}
        proj_at = {46: (0, [recv_g[0]]), 62: (1, [recv_g[1]]),
                   70: (2, [recv_q[0], recv_q[1]])}
        for i in range(min(LOOK, nstep)):
            emit_scores(*steps[i])
        for i in range(nstep):
            if i + LOOK < nstep:
                emit_scores(*steps[i + LOOK])
            emit_av(*steps[i])
            if i in a2a_at:
                emit_a2a(*a2a_at[i])
            if i in proj_at:
                emit_proj(*proj_at[i])
        emit_proj(3, [recv_q[2], recv_q[3]])

    nc.compile()
    return nc


_NC_CACHE = None


def _get_nc():
    global _NC_CACHE
    if _NC_CACHE is None:
        _NC_CACHE = _build()
    return _NC_CACHE


def _bf16(a):
    import ml_dtypes
    return np.ascontiguousarray(a.astype(ml_dtypes.bfloat16))


def _in_maps(x, W_attn, b_attn, W_proj, b_proj):
    x = np.ascontiguousarray(np.asarray(x, dtype=np.float32))
    W_attn = np.asarray(W_attn, dtype=np.float32)
    b_attn = np.asarray(b_attn, dtype=np.float32)
    W_proj = np.ascontiguousarray(np.asarray(W_proj, dtype=np.float32))
    b_proj = np.asarray(b_proj, dtype=np.float32)

    # device-friendly layouts: [p, c, ...] with contiguous inner runs
    xT = _bf16(x.reshape(T, DK, 128).transpose(2, 1, 0))   # [128, DK, T]
    wp16 = _bf16(W_proj.reshape(DK, 128, D).transpose(1, 0, 2))  # [128, DK, D]
    bp2 = np.ascontiguousarray(b_proj.reshape(1, D))
    maps = []
    for c in range(NCORE):
        lo = c * CW
        sl_q = slice(lo, lo + CW)
        sl_k = slice(D + lo, D + lo + CW)
        sl_v = slice(2 * D + lo, 2 * D + lo + CW)
        maps.append({
            "xT": xT,
            "wq": _bf16(W_attn[:, sl_q].reshape(DK, 128, CW).transpose(1, 0, 2)),
            "wk": _bf16(W_attn[:, sl_k].reshape(DK, 128, CW).transpose(1, 0, 2)),
            "wv": _bf16(W_attn[:, sl_v].reshape(DK, 128, CW).transpose(1, 0, 2)),
            "bqkv": np.ascontiguousarray(
                np.stack([b_attn[sl_q], b_attn[sl_k], b_attn[sl_v]])),
            "wp": wp16,
            "bp": bp2,
        })
    return maps


def _gather(results):
    # core j's out rows, g-major (4 chunks of 128):
    #   g=0/1: (b0, qh): s = qh*1024 + j*128 + t
    #   g=2:   (b1, qc0|qc1): row 256 + p*64 + t <-> s = p*512 + j*64 + t
    #   g=3:   (b1, qc2|qc3): row 384 + p*64 + t <-> s = 1024 + p*512 + j*64 + t
    full = np.empty((B, S, D), dtype=np.float32)
    for j, r in enumerate(results):
        o = np.asarray(r["out"]).reshape(NG, GT, D)
        for qh in range(2):
            s0 = qh * 1024 + j * GT
            full[0, s0:s0 + GT, :] = o[qh]
        for qc in range(NQC):
            g, p = 2 + qc // 2, qc % 2
            s0 = qc * 512 + j * (GT // 2)
            full[1, s0:s0 + GT // 2, :] = o[g, p * 64:(p + 1) * 64]
    return full


def kernel(x, W_attn, b_attn, W_proj, b_proj):
    nc = _get_nc()
    maps = _in_maps(x, W_attn, b_attn, W_proj, b_proj)
    res = run_bass_kernel_spmd(nc, maps, core_ids=list(range(NCORE)))
    return _gather(res.results)


def kernel_traced(x, W_attn, b_attn, W_proj, b_proj, **kw):
    """Same as kernel() but with NTFF tracing; returns (out, BassKernelResults)."""
    nc = _get_nc()
    maps = _in_maps(x, W_attn, b_attn, W_proj, b_proj)
    res = run_bass_kernel_spmd(nc, maps, core_ids=list(range(NCORE)), trace=True, **kw)
    return _gather(res.results), res


# revision 22
# speedup vs baseline: 1.0590x; 1.0590x over previous
"""Causal self-attention (B=2, S=2048, D=1024, H=16) on 8 TRN2 NeuronCores.

Sharding: tensor-parallel over heads (2 heads/core) for qkv+attention,
then chunked AllToAll to token-parallel (512 tokens/core) for the output
projection.

Per-core kernel (SPMD, identical program, per-core weight slices as inputs):
  1. qkv^T projection from host-pre-transposed, host-bf16-cast x^T:
       Q^T/K^T/V^T [128ch(2 heads x 64), 4096 tok] = W_slice^T @ x
     (bf16 matmuls, fp32 PSUM accumulation)
  2. V^T -> V via batched 128x128 PE transposes (both heads per transpose),
     ones column interleaved per head so AV also produces the softmax
     denominator.
  3. Attention as one flat (b, qc, kc) software pipeline:
     - score matmuls for the two local heads are emitted back-to-back; they
       occupy disjoint row-halves of the PE array (tile_position auto-derived
       from kT partition base 0/64) so they run CONCURRENTLY (2x).
     - exp on ScalarE with scale=1/sqrt(hd) fused; causal = block skipping +
       one triangle-mask multiply on diagonal blocks.
     - AV accumulates y^T (64 rows) + denominator row (ones col) in PSUM.
     - sender-side normalization: reciprocal of den row, broadcast via a
       col-packed pair of ones-matmuls (one PSUM bank, halves at partitions
       0:64/64:128), fused into the f32->bf16 cast of y.
  4. AllToAll in FOUR bf16 chunks (one per (batch, qc-pair)), each issued as
     soon as its attention slice is done -> overlapped with later attention.
  5. Output projection per chunk (128 tokens), interleaved between attention
     chunks so the PE stays dense/warm; out rows are g-major (g = 2b+qh).
Host gathers the 8 token-slices and reorders (see _gather).
"""

import numpy as np
from contextlib import ExitStack

import concourse.bass as bass
import concourse.bacc as bacc
import concourse.tile as tile
from concourse import mybir
from concourse.bass_utils import run_bass_kernel_spmd
from concourse.masks import make_identity

B, S, D = 2, 2048, 1024
H, HD = 16, 64
NCORE = 8
HPC = H // NCORE          # heads per core = 2
CW = HPC * HD             # channels per core = 128
T = B * S                 # 4096 tokens
TPC = T // NCORE          # 512 tokens per core (proj phase)
TCH = 512                 # token chunk for qkv projection
NT = T // TCH             # 8
QCH = 512                 # query chunk
KCH = 128                 # key chunk
NQC = S // QCH            # 4 query chunks per batch
NKC = S // KCH            # 16 key chunks per batch
DK = D // 128             # 8 contraction chunks of 128
NG = 4                    # A2A chunks: (b, qh)
GT = 128                  # tokens per core per chunk

f32 = mybir.dt.float32
f32r = mybir.dt.float32r
bf16 = mybir.dt.bfloat16
AF = mybir.ActivationFunctionType


def _build():
    nc = bacc.Bacc(None, target_bir_lowering=False, num_devices=NCORE)

    xT = nc.dram_tensor("xT", [128, DK, T], bf16, kind="ExternalInput")
    wq = nc.dram_tensor("wq", [128, DK, CW], bf16, kind="ExternalInput")
    wk = nc.dram_tensor("wk", [128, DK, CW], bf16, kind="ExternalInput")
    wv = nc.dram_tensor("wv", [128, DK, CW], bf16, kind="ExternalInput")
    bqkv = nc.dram_tensor("bqkv", [3, CW], f32, kind="ExternalInput")
    wp = nc.dram_tensor("wp", [128, DK, D], bf16, kind="ExternalInput")
    bp = nc.dram_tensor("bp", [1, D], f32, kind="ExternalInput")
    out = nc.dram_tensor("out", [TPC, D], f32, kind="ExternalOutput")

    with ExitStack() as ctx:
        tc = ctx.enter_context(tile.TileContext(nc))
        const = ctx.enter_context(tc.tile_pool(name="const", bufs=1))
        dram = ctx.enter_context(tc.tile_pool(name="dram", bufs=1, space="DRAM"))
        wqkv_pool = ctx.enter_context(tc.tile_pool(name="wqkv", bufs=1))
        xt_pool = ctx.enter_context(tc.tile_pool(name="xt", bufs=3))
        qkvt_pool = ctx.enter_context(tc.tile_pool(name="qkvt", bufs=1))
        wp_pool = ctx.enter_context(tc.tile_pool(name="wpp", bufs=1))
        vpool = ctx.enter_context(tc.tile_pool(name="vpool", bufs=2))
        ppool = ctx.enter_context(tc.tile_pool(name="ppool", bufs=8))
        ypool = ctx.enter_context(tc.tile_pool(name="ypool", bufs=4))
        rpool = ctx.enter_context(tc.tile_pool(name="rpool", bufs=4))
        rgpool = ctx.enter_context(tc.tile_pool(name="rgpool", bufs=2))
        opool = ctx.enter_context(tc.tile_pool(name="opool", bufs=2))
        ps_big = ctx.enter_context(tc.tile_pool(name="ps_big", bufs=2, space="PSUM"))
        ps_sc = ctx.enter_context(tc.tile_pool(name="ps_sc", bufs=2, space="PSUM"))
        ps_y = ctx.enter_context(tc.tile_pool(name="ps_y", bufs=2, space="PSUM"))

        # ---- constants ----
        identity = const.tile([128, 128], bf16)
        make_identity(nc, identity[:])
        # mask[k, q] = 1.0 if k <= q else 0.0  (keep lower-left in S^T layout)
        mask = const.tile([128, 128], bf16)
        nc.gpsimd.memset(mask[:], 0.0)
        nc.gpsimd.affine_select(
            out=mask[:], in_=mask[:],
            compare_op=mybir.AluOpType.is_ge,  # iota(k-q-1) >= 0 (k>q) -> keep 0; else fill 1
            fill=1.0, base=-1, pattern=[[-1, 128]], channel_multiplier=1,
        )
        ones_f32 = const.tile([128, 128], f32)
        nc.vector.memset(ones_f32[:], 1.0)
        ones_bf = const.tile([128, 32], bf16)
        nc.vector.memset(ones_bf[:], 1.0)
        ones_row = const.tile([1, 128], f32r)
        nc.vector.tensor_copy(ones_row[:], ones_f32[0:1, :])
        bias_sb = const.tile([128, 3], f32)
        nc.sync.dma_start(bias_sb[:], bqkv[:].rearrange("g p -> p g"))
        bp_sb = const.tile([1, D], f32r)
        nc.sync.dma_start(bp_sb[:], bp[:].bitcast(f32r))

        # ---- weights (bf16) ----
        wq_sb = wqkv_pool.tile([128, DK, CW], bf16)
        wk_sb = wqkv_pool.tile([128, DK, CW], bf16)
        wv_sb = wqkv_pool.tile([128, DK, CW], bf16)
        nc.sync.dma_start(wq_sb[:], wq[:])
        nc.sync.dma_start(wk_sb[:], wk[:])
        nc.sync.dma_start(wv_sb[:], wv[:])

        # ---- A2A buffers (bf16, 4 chunks of (b, qh)) ----
        SH = CW + 2  # 128 y rows (2 heads) + 2 denominator rows
        send_g = [dram.tile([NCORE, SH, GT], bf16, name=f"send{g}") for g in range(NG)]
        recv_g = [dram.tile([NCORE, SH, GT], bf16, name=f"recv{g}") for g in range(NG)]

        # ---- phase 2: batched V transposes (both heads per 128x128 block),
        # ones column interleaved per head (col 64 of each 65-col half) ----
        vboths = {}

        def emit_vtrans(b):
            vboth = vpool.tile([128, NKC, 2 * (HD + 1)], bf16, name=f"vboth{b}")
            vboths[b] = vboth
            nc.vector.tensor_copy(
                vboth[:].rearrange("p c (h x) -> p c h x", x=HD + 1)[:, :, :, HD],
                ones_bf[:].rearrange("p (c h) -> p c h", h=2),
            )
            VG = 4  # transposes batched per PSUM bank
            for g in range(NKC // VG):
                pst = ps_big.tile([128, VG, 128], bf16, tag="psbig")
                for u in range(VG):
                    kc = g * VG + u
                    nc.tensor.transpose(
                        pst[:, u, :],
                        vT[:, b * S + kc * KCH: b * S + (kc + 1) * KCH],
                        identity[:],
                    )
                nc.vector.tensor_copy(
                    vboth[:, g * VG:(g + 1) * VG, :]
                    .rearrange("p c (h x) -> p c h x", x=HD + 1)[:, :, :, 0:HD],
                    pst[:].rearrange("p c (h x) -> p c h x", x=HD),
                )

        # warmup collective on scratch (contents unused): absorbs NRT's
        # ~11.5us first-trigger latency + slow first transfer under qkv
        warm_s = dram.tile([NCORE, 1024], bf16, name="warm_s")
        warm_r = dram.tile([NCORE, 1024], bf16, name="warm_r")
        nc.gpsimd.collective_compute(
            "AllToAll", mybir.AluOpType.bypass,
            replica_groups=[list(range(NCORE))],
            ins=[warm_s[:].opt()], outs=[warm_r[:].opt()],
        )

        # ---- phase 1: qkv^T projection ----
        qT = qkvt_pool.tile([128, T], bf16)
        kT = qkvt_pool.tile([128, T], bf16)
        vT = qkvt_pool.tile([128, T], bf16)
        wp_sb = wp_pool.tile([128, DK, D], bf16)
        for tp in range(NT // 2):
            t0, t1 = 2 * tp, 2 * tp + 1
            xt = xt_pool.tile([128, DK, 2 * TCH], bf16, tag="xt")
            xr = xT[:]
            hk = DK // 2
            for ci in range(2):
                cs = slice(ci * hk, (ci + 1) * hk)
                nc.sync.dma_start(xt[:, cs, 0:TCH], xr[:, cs, t0 * TCH:(t0 + 1) * TCH])
                nc.sync.dma_start(xt[:, cs, TCH:2 * TCH], xr[:, cs, t1 * TCH:(t1 + 1) * TCH])
            if tp == 1:
                # wp needed only at proj; don't let it delay the first xt chunks
                nc.sync.dma_start(wp_sb[:], wp[:])
            for gi, (wsb, dst) in enumerate([(wq_sb, qT), (wk_sb, kT), (wv_sb, vT)]):
                ps0 = ps_big.tile([128, TCH], f32, tag="psbig")
                ps1 = ps_big.tile([128, TCH], f32, tag="psbig")
                for c in range(DK):
                    # consecutive matmuls share lhsT -> one weight load serves two
                    nc.tensor.matmul(
                        ps0[:], lhsT=wsb[:, c, :], rhs=xt[:, c, 0:TCH],
                        start=(c == 0), stop=(c == DK - 1),
                    )
                    nc.tensor.matmul(
                        ps1[:], lhsT=wsb[:, c, :], rhs=xt[:, c, TCH:2 * TCH],
                        start=(c == 0), stop=(c == DK - 1),
                    )
                for ti, ps in ((t0, ps0), (t1, ps1)):
                    nc.vector.tensor_scalar_add(
                        dst[:, ti * TCH:(ti + 1) * TCH], ps[:], bias_sb[:, gi:gi + 1])
            if t1 * TCH + TCH == S:
                emit_vtrans(0)
            elif t1 * TCH + TCH == 2 * S:
                emit_vtrans(1)

        # ---- phase 3+4+5: flat attention pipeline, chunked A2A, interleaved proj ----
        def geom(qc, kc):
            q0 = qc * QCH
            diag = kc * KCH >= q0
            koff = kc * KCH - q0 if diag else 0
            return koff, QCH - koff

        # flat step list: (b, qc, kc)
        steps = []
        for b in range(B):
            for qc in range(NQC):
                nkc = 4 * (qc + 1)
                for kc in range(nkc):
                    steps.append((b, qc, kc))

        ypss = {}   # (b, qc) -> [yps_h0, yps_h1]
        pts = {}    # (b, qc, kc, hl) -> pt tile

        def emit_scores(b, qc, kc):
            koff, W_ = geom(qc, kc)
            # both heads' scores into one 2-bank PSUM tile: the score MMs stay
            # adjacent (concurrent row-halves of the PE) and ONE ACTIVATE
            # covers both heads (halves the per-call exp overhead)
            sps = ps_sc.tile([128, HPC, QCH], f32, tag="sps")
            for hl in range(HPC):
                r0 = hl * HD
                nc.tensor.matmul(
                    sps[:, hl, 0:W_],
                    lhsT=kT[r0:r0 + HD, b * S + kc * KCH: b * S + (kc + 1) * KCH],
                    rhs=qT[r0:r0 + HD, b * S + qc * QCH + koff: b * S + (qc + 1) * QCH],
                    start=True, stop=True,
                )
            pt = ppool.tile([128, HPC, QCH], bf16, tag="pt")
            nc.scalar.activation(pt[:, :, 0:W_], sps[:, :, 0:W_], AF.Exp, scale=0.125)
            if kc * KCH >= qc * QCH:
                for hl in range(HPC):
                    nc.vector.tensor_mul(pt[:, hl, 0:KCH], pt[:, hl, 0:KCH], mask[:])
            pts[(b, qc, kc)] = pt

        def emit_av(b, qc, kc):
            koff, W_ = geom(qc, kc)
            nkc = 4 * (qc + 1)
            if kc == 0:
                ypss[(b, qc)] = [
                    ps_y.tile([HD + 1, QCH], f32, tag="yps", name=f"yps{b}{qc}{hl}")
                    for hl in range(HPC)
                ]
            vboth = vboths[b]
            last = kc == nkc - 1
            pt = pts.pop((b, qc, kc))
            for hl in range(HPC):
                nc.tensor.matmul(
                    ypss[(b, qc)][hl][:, koff:QCH],
                    lhsT=vboth[:, kc, hl * (HD + 1):(hl + 1) * (HD + 1)],
                    rhs=pt[:, hl, 0:W_],
                    start=(kc == 0), stop=last,
                )
            if last:
                # ship unnormalized y (bf16) + denominator rows; the receiver
                # normalizes (recip is cheap there: [16,128] once per chunk)
                yp = ypss.pop((b, qc))
                g = 2 * b + qc // 2
                j0 = (qc % 2) * 4
                for hl in range(HPC):
                    ysb = ypool.tile([HD + 1, QCH], bf16, tag="ysb", bufs=8)
                    nc.vector.tensor_copy(ysb[:], yp[hl][:])
                    # SBUF AP stays partition-major: transpose traversal on
                    # the DRAM side
                    nc.sync.dma_start(
                        send_g[g][j0:j0 + 4, hl * HD:(hl + 1) * HD, :]
                        .rearrange("j r t -> r j t"),
                        ysb[0:HD, :].rearrange("r (j t) -> r j t", t=GT),
                    )
                    nc.sync.dma_start(
                        send_g[g][j0:j0 + 4, CW + hl:CW + hl + 1, :]
                        .rearrange("j r t -> r j t"),
                        ysb[HD:HD + 1, :].rearrange("r (j t) -> r j t", t=GT),
                    )

        def emit_a2a(snd, rcv):
            nc.gpsimd.collective_compute(
                "AllToAll", mybir.AluOpType.bypass,
                replica_groups=[list(range(NCORE))],
                ins=[snd[:].opt()], outs=[rcv[:].opt()],
            )

        def emit_proj(g, recvs):
            HT = GT // len(recvs)
            rg = rgpool.tile([128, NCORE, GT], bf16, tag="rg")
            dden = rgpool.tile([2 * NCORE, GT], bf16, tag="dden")
            for p, rv in enumerate(recvs):
                nc.sync.dma_start(rg[:, :, p * HT:(p + 1) * HT],
                                  rv[:, 0:CW, :].rearrange("c p t -> p c t"))
                nc.sync.dma_start(dden[0:NCORE, p * HT:(p + 1) * HT], rv[:, CW, :])
                nc.sync.dma_start(dden[NCORE:2 * NCORE, p * HT:(p + 1) * HT],
                                  rv[:, CW + 1, :])
            rcp = rgpool.tile([2 * NCORE, GT], f32, tag="rcp")
            nc.vector.reciprocal(rcp[:], dden[:])
            # fold the 16 recip rows into one partition's free dim, then
            # broadcast to all 128 partitions in one gpsimd op
            rcpf = rgpool.tile([1, 2 * NCORE, GT], f32, tag="rcpf")
            nc.sync.dma_start(rcpf[:], rcp[:])
            sclb = rgpool.tile([128, 2 * NCORE, GT], f32, tag="sclb")
            nc.gpsimd.partition_broadcast(sclb[:], rcpf[0:1, :, :])
            rgn = rgpool.tile([128, NCORE, GT], bf16, tag="rgn")
            sv = sclb[:].rearrange("p (h c) t -> p h c t", c=NCORE)
            for hl in range(HPC):
                nc.vector.tensor_mul(
                    rgn[hl * HD:(hl + 1) * HD, :, :],
                    rg[hl * HD:(hl + 1) * HD, :, :],
                    sv[hl * HD:(hl + 1) * HD, hl, :, :],
                )
            pss = [ps_big.tile([128, 512], f32, tag="psbig", name=f"pso{g}{n}")
                   for n in range(D // 512)]
            for c in range(DK):
                for n in range(D // 512):
                    nc.tensor.matmul(
                        pss[n][:],
                        lhsT=rgn[:, c, :],
                        rhs=wp_sb[:, c, n * 512:(n + 1) * 512],
                        start=(c == 0), stop=False,
                    )
            for n in range(D // 512):
                nc.tensor.matmul(
                    pss[n][:], lhsT=ones_row[:],
                    rhs=bp_sb[:, n * 512:(n + 1) * 512],
                    start=False, stop=True,
                )
                osb = opool.tile([128, 512], f32, tag="osb")
                nc.vector.tensor_copy(osb[:], pss[n][:])
                nc.sync.dma_start(out[g * GT:(g + 1) * GT, n * 512:(n + 1) * 512], osb[:])

        LOOK = 2
        nstep = len(steps)
        # A2A fires at each chunk's last step; proj#g emitted once its A2A
        # should have landed (proj#3 at the end)
        a2a_at = {11: (send_g[0], recv_g[0]), 39: (send_g[1], recv_g[1]),
                  51: (send_g[2], recv_g[2]), 79: (send_g[3], recv_g[3])}
        proj_at = {52: (0, [recv_g[0]]), 62: (1, [recv_g[1]]),
                   72: (2, [recv_g[2]])}
        for i in range(min(LOOK, nstep)):
            emit_scores(*steps[i])
        for i in range(nstep):
            if i + LOOK < nstep:
                emit_scores(*steps[i + LOOK])
            emit_av(*steps[i])
            if i in a2a_at:
                emit_a2a(*a2a_at[i])
            if i in proj_at:
                emit_proj(*proj_at[i])
        # filler matmuls on constants: keep the PE activity monitor warm
        # through the final A2A wait so proj#3 runs at full clock
        for f in range(40):
            fps = ps_big.tile([128, 512], f32, tag="psbig")
            nc.tensor.matmul(fps[0:64, 0:128], lhsT=identity[0:64, 0:64],
                             rhs=mask[0:64, :], start=True, stop=True)
        emit_proj(3, [recv_g[3]])

    nc.compile()
    return nc


_NC_CACHE = None


def _get_nc():
    global _NC_CACHE
    if _NC_CACHE is None:
        _NC_CACHE = _build()
    return _NC_CACHE


def _bf16(a):
    import ml_dtypes
    return np.ascontiguousarray(a.astype(ml_dtypes.bfloat16))


def _in_maps(x, W_attn, b_attn, W_proj, b_proj):
    x = np.ascontiguousarray(np.asarray(x, dtype=np.float32))
    W_attn = np.asarray(W_attn, dtype=np.float32)
    b_attn = np.asarray(b_attn, dtype=np.float32)
    W_proj = np.ascontiguousarray(np.asarray(W_proj, dtype=np.float32))
    b_proj = np.asarray(b_proj, dtype=np.float32)

    # device-friendly layouts: [p, c, ...] with contiguous inner runs
    xT = _bf16(x.reshape(T, DK, 128).transpose(2, 1, 0))   # [128, DK, T]
    wp16 = _bf16(W_proj.reshape(DK, 128, D).transpose(1, 0, 2))  # [128, DK, D]
    bp2 = np.ascontiguousarray(b_proj.reshape(1, D))
    maps = []
    for c in range(NCORE):
        lo = c * CW
        sl_q = slice(lo, lo + CW)
        sl_k = slice(D + lo, D + lo + CW)
        sl_v = slice(2 * D + lo, 2 * D + lo + CW)
        maps.append({
            "xT": xT,
            "wq": _bf16(W_attn[:, sl_q].reshape(DK, 128, CW).transpose(1, 0, 2)),
            "wk": _bf16(W_attn[:, sl_k].reshape(DK, 128, CW).transpose(1, 0, 2)),
            "wv": _bf16(W_attn[:, sl_v].reshape(DK, 128, CW).transpose(1, 0, 2)),
            "bqkv": np.ascontiguousarray(
                np.stack([b_attn[sl_q], b_attn[sl_k], b_attn[sl_v]])),
            "wp": wp16,
            "bp": bp2,
        })
    return maps


def _gather(results):
    # core j's out rows are g-major: row = g*GT + t, g = 2*b + qh;
    # global token (b, s) with s = qh*1024 + j*128 + t
    full = np.empty((B, S, D), dtype=np.float32)
    for j, r in enumerate(results):
        o = np.asarray(r["out"]).reshape(NG, GT, D)
        for b in range(B):
            for qh in range(2):
                g = 2 * b + qh
                s0 = qh * 1024 + j * GT
                full[b, s0:s0 + GT, :] = o[g]
    return full


def kernel(x, W_attn, b_attn, W_proj, b_proj):
    nc = _get_nc()
    maps = _in_maps(x, W_attn, b_attn, W_proj, b_proj)
    res = run_bass_kernel_spmd(nc, maps, core_ids=list(range(NCORE)))
    return _gather(res.results)


def kernel_traced(x, W_attn, b_attn, W_proj, b_proj, **kw):
    """Same as kernel() but with NTFF tracing; returns (out, BassKernelResults)."""
    nc = _get_nc()
    maps = _in_maps(x, W_attn, b_attn, W_proj, b_proj)
    res = run_bass_kernel_spmd(nc, maps, core_ids=list(range(NCORE)), trace=True, **kw)
    return _gather(res.results), res
